# revision 1
# baseline (speedup 1.0000x reference)
"""MoE BatchedExperts kernel for 8 trn2 NeuronCores.

Strategy: expert parallelism with host-side top-k dispatch. Each token has
exactly TOP_K nonzero routing weights, so core e only processes the tokens
routed to expert e (~N*K/E of them) instead of all N — 4x less compute than
the dense reference formulation, identical math (zero-score tokens
contribute zero).

Per core e (tokens gathered+transposed on host to xT [D, T]):
  hT = gelu(mm1 + b0)   [F, T]   mm1: lhsT=w0 chunk [128,128], rhs=xT chunk
  y  = hT.T @ w1[e]     [T, D]   mm2: lhsT=hT chunk [128,128], rhs=w1 chunk
Host combines: out[idx_e] += r_e * y_e rows; b1 folded in via routing @ b1.

All matmuls run as float32r (tf32-like, ~1e-4 rel err, full PE rate:
1 cycle/row warm). PE is the bottleneck; everything else overlaps.
"""

import numpy as np

import concourse.bacc as bacc
import concourse.mybir as mybir
from concourse.tile import TileContext
from concourse.bass_utils import run_bass_kernel_spmd

F32 = mybir.dt.float32
F32R = mybir.dt.float32r

N, D, E, F = 4096, 1024, 8, 2048
P = 128
KD = D // P            # 8  k-tiles for mm1
KF = F // P            # 16 k-tiles for mm2
T_CHUNKS = [256, 384, 512]   # mm1 moving-dim chunks (>=256 keeps fp32r full rate)
TCH = 384              # token pad granularity
D_CHUNKS = [512, 512]        # mm2 moving-dim chunks (sum = D)
assert sum(D_CHUNKS) == D
KH = KF // 2                 # w1 streamed in (dc, k-half) tiles

_cache: dict[int, object] = {}


def build_program(T: int):
    """Bass program for one expert shard with T padded tokens."""
    assert T % TCH == 0 and T % P == 0
    TO = T // P
    # graduated chunk sizes: small first chunk -> earlier PE start
    if T == 1152:
        t_chunks = list(T_CHUNKS)
    else:
        t_chunks = [TCH] * (T // TCH)
    NTC = len(t_chunks)
    t_offs = [0, *np.cumsum(t_chunks).tolist()]

    nc = bacc.Bacc("TRN2", target_bir_lowering=False, debug=False)
    xT = nc.dram_tensor("xT", [D, T], F32R, kind="ExternalInput")
    w0 = nc.dram_tensor("w0", [D, F], F32R, kind="ExternalInput")
    w1 = nc.dram_tensor("w1", [F, D], F32R, kind="ExternalInput")
    # b0 comes pre-arranged [128, KF] on the host so the DMA is contiguous
    # 64B runs per partition (a [F]-strided load is 2048 4-byte descriptors
    # that clog the ring for ~10us)
    b0 = nc.dram_tensor("b0", [P, KF], F32, kind="ExternalInput")
    y = nc.dram_tensor("y", [T, D], F32, kind="ExternalOutput")

    xT_r = xT.rearrange("(ko p) t -> p ko t", p=P)
    w0_r = w0.rearrange("(ko p) f -> p ko f", p=P)
    w1_r = w1.rearrange("(ko p) d -> p ko d", p=P)

    with TileContext(nc) as tc:
        with tc.tile_pool(name="const", bufs=1) as const, \
             tc.tile_pool(name="xpool", bufs=1) as xpool, \
             tc.tile_pool(name="hpool", bufs=1) as hpool, \
             tc.tile_pool(name="w0pool", bufs=5) as w0pool, \
             tc.tile_pool(name="w1pool", bufs=4) as w1pool, \
             tc.tile_pool(name="ypool", bufs=3) as ypool, \
             tc.tile_pool(name="psum", bufs=8, space="PSUM") as psum:

            # x resident in SBUF at the head of the scalar ring (before any
            # gelu ACT and before b0 — few big DMAs: many small ones hit
            # HWDGE semaphore-reuse round-trips); w0 owns the sync ring
            x_sb = []
            for t in range(NTC):
                xt = xpool.tile([P, KD, t_chunks[t]], F32R, tag=f"x{t}",
                                name=f"x{t}")
                nc.scalar.dma_start(xt[:], xT_r[:, :, t_offs[t]:t_offs[t + 1]])
                x_sb.append(xt)

            b0_sb = const.tile([P, KF], F32)
            nc.scalar.dma_start(b0_sb[:], b0[:, :])

            # hT = gelu(x @ w0 + b0), laid out [F-part, T-free], fp32r
            h_sb = hpool.tile([P, KF, T], F32R)

            # w1 (dc, k-half) tiles; DMAs dripped into the scalar stream
            # mid-phase-1 (behind gelu ACTs) so they can't steal bandwidth
            # from the ramp. The last shares a slot with the first and is
            # issued in phase 2 once the slot frees.
            w1_sb = {}
            for dc in range(len(D_CHUNKS)):
                for kh in range(2):
                    w1_sb[dc, kh] = w1pool.tile([P, KH, 512], F32R, tag="w1",
                                                name=f"w1_{dc}_{kh}")

            def load_w1(engine, dc, kh):
                engine.dma_start(
                    w1_sb[dc, kh][:],
                    w1_r[:, kh * KH:(kh + 1) * KH, dc * 512:(dc + 1) * 512])

            # ---- phase 1: mm1 + gelu ----
            # (fo; t; k): the first psum group needs only xT chunk t0, so the
            # PE starts while t1/t2 are still arriving
            # first three fo interleaved t-major: 3x more PE work available
            # per arrived xT chunk during the ramp
            pairs = [(f, t) for t in range(NTC) for f in range(3)]
            pairs += [(f, t) for f in range(3, KF) for t in range(NTC)]
            gate_tile = const.tile([P, 1], F32R, name="gate_tile")
            w0_tiles = {}
            for gi, (fo, t) in enumerate(pairs):
                w0_sb = w0_tiles.get(fo)
                if w0_sb is None:
                    if fo == 3:
                        # tiny SBUF->SBUF dummy reading xT chunk 0: FIFO holds
                        # the sync ring here until t0 lands, so deep w0
                        # prefetch can't bandwidth-starve the first chunk
                        nc.sync.dma_start(gate_tile[:], x_sb[0][:, 0, 0:1])
                    w0_sb = w0_tiles[fo] = w0pool.tile([P, KD, P], F32R,
                                                       tag="w0", name=f"w0_{fo}")
                    nc.sync.dma_start(w0_sb[:], w0_r[:, :, fo * P:(fo + 1) * P])
                ps = psum.tile([P, 512], F32, tag="ps",
                               name=f"ps1_{fo}_{t}")[:, :t_chunks[t]]
                for k in range(KD):
                    nc.tensor.matmul(ps, w0_sb[:, k], x_sb[t][:, k],
                                     start=(k == 0), stop=(k == KD - 1))
                nc.scalar.activation(h_sb[:, fo, t_offs[t]:t_offs[t + 1]], ps,
                                     mybir.ActivationFunctionType.Gelu,
                                     bias=b0_sb[:, fo:fo + 1])
                # drip the w1 loads into the scalar stream mid-phase-1 so
                # they don't compete with xT/w0 during the ramp
                if t == 2 and fo in (6, 8, 10, 12):
                    dc, kh = [(0, 0), (0, 1), (1, 0), (1, 1)][(fo - 6) // 2]
                    load_w1(nc.scalar, dc, kh)

            # ---- phase 2: mm2 ----
            for dc, DCH in enumerate(D_CHUNKS):
                for to in range(TO):
                    ps2 = psum.tile([P, 512], F32, tag="ps",
                                    name=f"ps2_{dc}_{to}")
                    for k in range(KF):
                        nc.tensor.matmul(ps2, h_sb[:, k, to * P:(to + 1) * P],
                                         w1_sb[dc, k // KH][:, k % KH],
                                         start=(k == 0), stop=(k == KF - 1))
                    y_sb = ypool.tile([P, 512], F32, tag="y",
                                      name=f"y_{dc}_{to}")
                    nc.vector.tensor_copy(y_sb[:], ps2)
                    nc.sync.dma_start(
                        y[to * P:(to + 1) * P, dc * 512:(dc + 1) * 512], y_sb[:])

    nc.compile()
    return nc


def kernel(x, routing_tensor, w0, b0, w1, b1):
    x = np.ascontiguousarray(np.asarray(x, dtype=np.float32))
    routing = np.asarray(routing_tensor, dtype=np.float32)
    w0 = np.ascontiguousarray(np.asarray(w0, dtype=np.float32))
    b0 = np.asarray(b0, dtype=np.float32)
    w1 = np.ascontiguousarray(np.asarray(w1, dtype=np.float32))
    b1 = np.asarray(b1, dtype=np.float32)

    idx = [np.nonzero(routing[:, e])[0] for e in range(E)]
    cnt = [len(i) for i in idx]
    T = max(TCH, -(-max(cnt) // TCH) * TCH)

    nc = _cache.get(T)
    if nc is None:
        nc = _cache[T] = build_program(T)

    in_maps = []
    for e in range(E):
        xTe = np.zeros((D, T), dtype=np.float32)
        xTe[:, :cnt[e]] = x[idx[e]].T
        b0e = np.ascontiguousarray(b0[e, 0].reshape(KF, P).T)
        in_maps.append({"xT": xTe, "w0": w0[e], "w1": w1[e], "b0": b0e})

    res = run_bass_kernel_spmd(nc, in_maps, core_ids=list(range(E)))

    # combine: out = sum_e r_e * (y_e + b1_e)
    out = routing @ b1[:, 0, :]
    for e in range(E):
        r = routing[idx[e], e:e + 1]
        out[idx[e]] += r * res.results[e]["y"][:cnt[e]]
    return out.astype(np.float32)



# revision 2
# speedup vs baseline: 1.1015x; 1.1015x over previous
"""MoE BatchedExperts kernel for 8 trn2 NeuronCores.

Strategy: expert parallelism with host-side top-k dispatch. Each token has
exactly TOP_K nonzero routing weights, so core e only processes the tokens
routed to expert e (~N*K/E of them) instead of all N — 4x less compute than
the dense reference formulation, identical math (zero-score tokens
contribute zero).

Per core e (tokens gathered+transposed on host to xT [D, T]):
  hT = gelu(mm1 + b0)   [F, T]   mm1: lhsT=w0 chunk [128,128], rhs=xT chunk
  y  = hT.T @ w1[e]     [T, D]   mm2: lhsT=hT chunk [128,128], rhs=w1 chunk
Host combines: out[idx_e] += r_e * y_e rows; b1 folded in via routing @ b1.

All matmuls run as float32r (tf32-like, ~1e-4 rel err, full PE rate:
1 cycle/row warm). PE is the bottleneck; everything else overlaps.
"""

import numpy as np
import ml_dtypes

import concourse.bacc as bacc
import concourse.mybir as mybir
from concourse.tile import TileContext
from concourse.bass_utils import run_bass_kernel_spmd

F32 = mybir.dt.float32
BF16 = mybir.dt.bfloat16

N, D, E, F = 4096, 1024, 8, 2048
P = 128
KD = D // P            # 8  k-tiles for mm1
KF = F // P            # 16 k-tiles for mm2
T_CHUNKS = [256, 384, 512]   # mm1 moving-dim chunks (>=256 keeps fp32r full rate)
TCH = 384              # token pad granularity
D_CHUNKS = [512, 512]        # mm2 moving-dim chunks (sum = D)
assert sum(D_CHUNKS) == D
KH = KF // 2                 # w1 streamed in (dc, k-half) tiles

_cache: dict[int, object] = {}


def build_program(T: int):
    """Bass program for one expert shard with T padded tokens."""
    assert T % TCH == 0 and T % P == 0
    TO = T // P
    # graduated chunk sizes: small first chunk -> earlier PE start
    if T == 1152:
        t_chunks = list(T_CHUNKS)
    else:
        t_chunks = [TCH] * (T // TCH)
    NTC = len(t_chunks)
    t_offs = [0, *np.cumsum(t_chunks).tolist()]

    nc = bacc.Bacc("TRN2", target_bir_lowering=False, debug=False)
    xT = nc.dram_tensor("xT", [D, T], BF16, kind="ExternalInput")
    w0 = nc.dram_tensor("w0", [D, F], BF16, kind="ExternalInput")
    w1 = nc.dram_tensor("w1", [F, D], BF16, kind="ExternalInput")
    # b0 comes pre-arranged [128, KF] on the host so the DMA is contiguous
    # 64B runs per partition (a [F]-strided load is 2048 4-byte descriptors
    # that clog the ring for ~10us)
    b0 = nc.dram_tensor("b0", [P, KF], F32, kind="ExternalInput")
    y = nc.dram_tensor("y", [T, D], F32, kind="ExternalOutput")

    xT_r = xT.rearrange("(ko p) t -> p ko t", p=P)
    w0_r = w0.rearrange("(ko p) f -> p ko f", p=P)
    w1_r = w1.rearrange("(ko p) d -> p ko d", p=P)

    with TileContext(nc) as tc:
        with tc.tile_pool(name="const", bufs=1) as const, \
             tc.tile_pool(name="xpool", bufs=1) as xpool, \
             tc.tile_pool(name="hpool", bufs=1) as hpool, \
             tc.tile_pool(name="w0pool", bufs=5) as w0pool, \
             tc.tile_pool(name="w1pool", bufs=4) as w1pool, \
             tc.tile_pool(name="ypool", bufs=3) as ypool, \
             tc.tile_pool(name="psum", bufs=8, space="PSUM") as psum:

            # x resident in SBUF at the head of the scalar ring (before any
            # gelu ACT and before b0 — few big DMAs: many small ones hit
            # HWDGE semaphore-reuse round-trips); w0 owns the sync ring
            x_sb = []
            for t in range(NTC):
                xt = xpool.tile([P, KD, t_chunks[t]], BF16, tag=f"x{t}",
                                name=f"x{t}")
                nc.scalar.dma_start(xt[:], xT_r[:, :, t_offs[t]:t_offs[t + 1]])
                x_sb.append(xt)

            b0_sb = const.tile([P, KF], F32)
            nc.scalar.dma_start(b0_sb[:], b0[:, :])

            # hT = gelu(x @ w0 + b0), laid out [F-part, T-free], fp32r
            h_sb = hpool.tile([P, KF, T], BF16)

            # w1 (dc, k-half) tiles; DMAs dripped into the scalar stream
            # mid-phase-1 (behind gelu ACTs) so they can't steal bandwidth
            # from the ramp. The last shares a slot with the first and is
            # issued in phase 2 once the slot frees.
            w1_sb = {}
            for dc in range(len(D_CHUNKS)):
                for kh in range(2):
                    w1_sb[dc, kh] = w1pool.tile([P, KH, 512], BF16, tag="w1",
                                                name=f"w1_{dc}_{kh}")

            def load_w1(engine, dc, kh):
                engine.dma_start(
                    w1_sb[dc, kh][:],
                    w1_r[:, kh * KH:(kh + 1) * KH, dc * 512:(dc + 1) * 512])

            # ---- phase 1: mm1 + gelu ----
            # (fo; t; k): the first psum group needs only xT chunk t0, so the
            # PE starts while t1/t2 are still arriving
            # first three fo interleaved t-major: 3x more PE work available
            # per arrived xT chunk during the ramp
            pairs = [(f, t) for t in range(NTC) for f in range(3)]
            pairs += [(f, t) for f in range(3, KF) for t in range(NTC)]
            gate_tile = const.tile([P, 1], BF16, name="gate_tile")
            w0_tiles = {}
            for gi, (fo, t) in enumerate(pairs):
                w0_sb = w0_tiles.get(fo)
                if w0_sb is None:
                    if fo == 3:
                        # tiny SBUF->SBUF dummy reading xT chunk 0: FIFO holds
                        # the sync ring here until t0 lands, so deep w0
                        # prefetch can't bandwidth-starve the first chunk
                        nc.sync.dma_start(gate_tile[:], x_sb[0][:, 0, 0:1])
                    w0_sb = w0_tiles[fo] = w0pool.tile([P, KD, P], BF16,
                                                       tag="w0", name=f"w0_{fo}")
                    nc.sync.dma_start(w0_sb[:], w0_r[:, :, fo * P:(fo + 1) * P])
                ps = psum.tile([P, 512], F32, tag="ps",
                               name=f"ps1_{fo}_{t}")[:, :t_chunks[t]]
                for k in range(KD):
                    nc.tensor.matmul(ps, w0_sb[:, k], x_sb[t][:, k],
                                     start=(k == 0), stop=(k == KD - 1))
                nc.scalar.activation(h_sb[:, fo, t_offs[t]:t_offs[t + 1]], ps,
                                     mybir.ActivationFunctionType.Gelu,
                                     bias=b0_sb[:, fo:fo + 1])
                # drip the w1 loads into the scalar stream mid-phase-1 so
                # they don't compete with xT/w0 during the ramp
                if t == 2 and fo in (6, 8, 10, 12):
                    dc, kh = [(0, 0), (0, 1), (1, 0), (1, 1)][(fo - 6) // 2]
                    load_w1(nc.scalar, dc, kh)

            # ---- phase 2: mm2 ----
            for dc, DCH in enumerate(D_CHUNKS):
                for to in range(TO):
                    ps2 = psum.tile([P, 512], F32, tag="ps",
                                    name=f"ps2_{dc}_{to}")
                    for k in range(KF):
                        nc.tensor.matmul(ps2, h_sb[:, k, to * P:(to + 1) * P],
                                         w1_sb[dc, k // KH][:, k % KH],
                                         start=(k == 0), stop=(k == KF - 1))
                    y_sb = ypool.tile([P, 512], F32, tag="y",
                                      name=f"y_{dc}_{to}")
                    nc.vector.tensor_copy(y_sb[:], ps2)
                    nc.sync.dma_start(
                        y[to * P:(to + 1) * P, dc * 512:(dc + 1) * 512], y_sb[:])

    nc.compile()
    return nc


def kernel(x, routing_tensor, w0, b0, w1, b1):
    x = np.ascontiguousarray(np.asarray(x, dtype=np.float32))
    routing = np.asarray(routing_tensor, dtype=np.float32)
    w0 = np.ascontiguousarray(np.asarray(w0, dtype=np.float32))
    b0 = np.asarray(b0, dtype=np.float32)
    w1 = np.ascontiguousarray(np.asarray(w1, dtype=np.float32))
    b1 = np.asarray(b1, dtype=np.float32)

    idx = [np.nonzero(routing[:, e])[0] for e in range(E)]
    cnt = [len(i) for i in idx]
    T = max(TCH, -(-max(cnt) // TCH) * TCH)

    nc = _cache.get(T)
    if nc is None:
        nc = _cache[T] = build_program(T)

    in_maps = []
    for e in range(E):
        xTe = np.zeros((D, T), dtype=ml_dtypes.bfloat16)
        xTe[:, :cnt[e]] = x[idx[e]].T.astype(ml_dtypes.bfloat16)
        b0e = np.ascontiguousarray(b0[e, 0].reshape(KF, P).T)
        in_maps.append({"xT": xTe, "w0": w0[e].astype(ml_dtypes.bfloat16), "w1": w1[e].astype(ml_dtypes.bfloat16), "b0": b0e})

    res = run_bass_kernel_spmd(nc, in_maps, core_ids=list(range(E)))

    # combine: out = sum_e r_e * (y_e + b1_e)
    out = routing @ b1[:, 0, :]
    for e in range(E):
        r = routing[idx[e], e:e + 1]
        out[idx[e]] += r * res.results[e]["y"][:cnt[e]]
    return out.astype(np.float32)



# revision 4
# speedup vs baseline: 1.1298x; 1.0257x over previous
"""MoE BatchedExperts kernel for 8 trn2 NeuronCores.

Strategy: expert parallelism with host-side top-k dispatch and exact load
balancing. Each token has TOP_K=2 nonzero routing weights; core c processes
a fixed per-core "slot structure" of expert token groups chosen so all
cores get ~N*K/E tokens (the hot experts are split across cores). All
matmuls run bf16 (1 row/cycle, same as fp32r, but half the DMA/SBUF and no
min-moving-dim constraint), PSUM accumulates fp32; measured end-to-end
rel err ~3e-3 vs the fp64 reference (gate 2e-2).

Per core, per group g (tokens gathered+transposed on host to xT [D, S_g]):
  h  = gelu(w0_g^T-tiles @ xT + b0)   [F-part, S_g]  tokens on moving dim
  yT = w1_g-tiles @ h                 [D-part, S_g]  tokens on moving dim
Host combines: out[idx] += r * yT.T rows; b1 folded in via routing @ b1.

Tokens stay on the PE moving dim in both phases so group sizes need no
128-padding. A few zero-filled warmup matmuls keep the PE busy (and ramp
its DVFS p-state) while the first input DMAs land.
"""

import numpy as np
import ml_dtypes

import concourse.bacc as bacc
import concourse.mybir as mybir
from concourse.tile import TileContext
from concourse.bass_utils import run_bass_kernel_spmd

F32 = mybir.dt.float32
BF16 = mybir.dt.bfloat16

N, D, E, F = 4096, 1024, 8, 2048
P = 128
KD = D // P            # 8  k-tiles for mm1 (contract D)
KF = F // P            # 16 k-tiles for mm2 (contract F)
DO = D // P            # 8  output d-tiles for mm2

_cache: dict[tuple, object] = {}


def _chunks_of(size: int) -> list[int]:
    """Split a group into near-equal moving-dim chunks <=512 (>=~250 keeps
    the per-matmul LDWEIGHTS (~97ns) hidden behind the previous matmul)."""
    n = -(-size // 512)
    base, rem = divmod(size, n)
    return [base + 1] * rem + [base] * (n - rem)


def build_program(sizes: tuple[int, ...]):
    """Bass program for one core: len(sizes) expert groups of fixed widths."""
    G = len(sizes)
    T = sum(sizes)
    goffs = [0, *np.cumsum(sizes).tolist()]
    chunks = [_chunks_of(s) for s in sizes]

    nc = bacc.Bacc("TRN2", target_bir_lowering=False, debug=False)
    xT = nc.dram_tensor("xT", [D, T], BF16, kind="ExternalInput")
    w0d = [nc.dram_tensor(f"w0_{g}", [D, F], BF16, kind="ExternalInput")
           for g in range(G)]
    w1d = [nc.dram_tensor(f"w1_{g}", [F, D], BF16, kind="ExternalInput")
           for g in range(G)]
    # b0 pre-arranged [128, G*KF] on the host (contiguous per-partition rows)
    b0 = nc.dram_tensor("b0", [P, G * KF], F32, kind="ExternalInput")
    yT = nc.dram_tensor("yT", [D, T], F32, kind="ExternalOutput")

    xT_r = xT.rearrange("(ko p) t -> p ko t", p=P)
    w0_r = [w.rearrange("(ko p) f -> p ko f", p=P) for w in w0d]
    w1_r = [w.rearrange("(ko p) d -> p ko d", p=P) for w in w1d]

    with TileContext(nc) as tc:
        with tc.tile_pool(name="const", bufs=1) as const, \
             tc.tile_pool(name="xpool", bufs=1) as xpool, \
             tc.tile_pool(name="hpool", bufs=1) as hpool, \
             tc.tile_pool(name="wpool", bufs=G + 1) as wpool, \
             tc.tile_pool(name="ypool", bufs=3) as ypool, \
             tc.tile_pool(name="psum", bufs=8, space="PSUM") as psum:

            # ---- PE warmup: zero matmuls ramp the DVFS p-state while the
            # first input DMAs are still in flight ----
            warm = const.tile([P, 512], BF16, name="warm")
            nc.vector.memset(warm[:], 0.0)
            for i in range(6):
                pw = psum.tile([P, 512], F32, tag="ps", name=f"warm{i}")
                nc.tensor.matmul(pw, warm[:, 0:P], warm[:], start=True,
                                 stop=True)

            # ---- input DMAs ----
            # scalar queue: b0, then x chunks in consumption order
            b0_sb = const.tile([P, G * KF], F32, name="b0")
            nc.scalar.dma_start(b0_sb[:], b0[:, :])
            x_sb = xpool.tile([P, KD, T], BF16, name="x")
            for g in range(G):
                off = goffs[g]
                for c in chunks[g]:
                    nc.scalar.dma_start(x_sb[:, :, off:off + c],
                                        xT_r[:, :, off:off + c])
                    off += c

            # sync queue: w0 per group, sliced so the first fo tiles land
            # fast; w1 streamed behind on the scalar queue. w0/w1 tiles
            # share G+1 slots (32KB each): w1 of the last group reuses the
            # slot w0 of group 0 releases after phase-1.
            w0_sb, w1_sb = [], []
            for g in range(G):
                w = wpool.tile([P, KD, F], BF16, tag="wbig", name=f"w0_{g}")
                w0_sb.append(w)
                fo_slices = ([0, 128, 512, 1024, 2048] if g == 0
                             else [0, 1024, 2048])
                for a, b in zip(fo_slices, fo_slices[1:]):
                    nc.sync.dma_start(w[:, :, a:b], w0_r[g][:, :, a:b])
            for g in range(G):
                w = wpool.tile([P, KF, D], BF16, tag="wbig", name=f"w1_{g}")
                w1_sb.append(w)
                for a in (0, 512):
                    nc.scalar.dma_start(w[:, :, a:a + 512],
                                        w1_r[g][:, :, a:a + 512])

            # h = gelu(x @ w0 + b0), [F-part, T-free], groups concatenated
            h_sb = hpool.tile([P, KF, T], BF16, name="h")

            # ---- phase 1: mm1 + gelu ----
            for g in range(G):
                off = goffs[g]
                for c in chunks[g]:
                    for fo in range(KF):
                        ps = psum.tile([P, 512], F32, tag="ps",
                                       name=f"ps1_{g}_{off}_{fo}")[:, :c]
                        for k in range(KD):
                            nc.tensor.matmul(
                                ps, w0_sb[g][:, k, fo * P:(fo + 1) * P],
                                x_sb[:, k, off:off + c],
                                start=(k == 0), stop=(k == KD - 1))
                        nc.scalar.activation(
                            h_sb[:, fo, off:off + c], ps,
                            mybir.ActivationFunctionType.Gelu,
                            bias=b0_sb[:, g * KF + fo:g * KF + fo + 1])
                    off += c

            # ---- phase 2: mm2 ----
            for g in range(G):
                off = goffs[g]
                for c in chunks[g]:
                    for do in range(DO):
                        ps2 = psum.tile([P, 512], F32, tag="ps",
                                        name=f"ps2_{g}_{off}_{do}")[:, :c]
                        for k in range(KF):
                            nc.tensor.matmul(
                                ps2, w1_sb[g][:, k, do * P:(do + 1) * P],
                                h_sb[:, k, off:off + c],
                                start=(k == 0), stop=(k == KF - 1))
                        y_sb = ypool.tile([P, 512], F32, tag="y",
                                          name=f"y_{g}_{off}_{do}")[:, :c]
                        nc.vector.tensor_copy(y_sb, ps2)
                        nc.sync.dma_start(
                            yT[do * P:(do + 1) * P, off:off + c], y_sb)
                    off += c

    nc.compile()
    return nc


def _plan(counts):
    """Choose per-core slot sizes (S1, S2) and assign expert token pieces.

    Minimizes T = S1 + S2 such that the 8 experts can be covered by 8
    pieces of size <= S1 plus 8 of size <= S2 (pieces of one expert may
    live on different cores). Falls back to one-slot-per-core (pure expert
    parallelism) if the search fails.
    """
    cmax = int(max(counts))
    order = sorted(range(E), key=lambda e: -counts[e])
    csort = [int(counts[e]) for e in order]

    def assign(S1, S2):
        from functools import lru_cache

        @lru_cache(maxsize=None)
        def feas(i, a, b):
            if i == len(csort):
                return ()
            c = csort[i]
            opts = []
            if c <= S1: opts.append((1, 0))
            if c <= S2: opts.append((0, 1))
            if c <= 2 * S2: opts.append((0, 2))
            if c <= S1 + S2: opts.append((1, 1))
            if c <= 2 * S1: opts.append((2, 0))
            if c <= S1 + 2 * S2: opts.append((1, 2))
            if c <= 2 * S1 + S2: opts.append((2, 1))
            opts.sort(key=lambda uv: (uv[0] + uv[1], S1 * uv[0] + S2 * uv[1]))
            for u, v in opts:
                if u <= a and v <= b:
                    rest = feas(i + 1, a - u, b - v)
                    if rest is not None:
                        return ((u, v),) + rest
            return None

        return feas(0, 8, 8)

    best = None
    for T in range(-(-N * 2 // E), cmax + 1):
        for S1 in range(-(-T // 2), T):
            S2 = T - S1
            sol = assign(S1, S2)
            if sol is not None:
                best = (S1, S2, sol)
                break
        if best:
            break
    if best is None:
        sizes = (cmax,)
        cores = [[(e, 0, int(counts[e]))] for e in range(E)]
        return sizes, cores

    S1, S2, sol = best
    s1_pieces, s2_pieces = [], []
    for i, (u, v) in enumerate(sol):
        e, c = order[i], csort[i]
        caps = [S1] * u + [S2] * v
        lo_ = 0
        for j, cap in enumerate(caps):
            take = min(cap, c - lo_)
            # ensure later pieces aren't left with more than they can hold
            take = max(take, c - lo_ - sum(caps[j + 1:]))
            (s1_pieces if cap == S1 else s2_pieces).append((e, lo_, take))
            lo_ += take
    while len(s1_pieces) < 8:
        s1_pieces.append((0, 0, 0))
    while len(s2_pieces) < 8:
        s2_pieces.append((0, 0, 0))
    sizes = (S1, S2)
    cores = [[s1_pieces[i], s2_pieces[i]] for i in range(8)]
    return sizes, cores


def kernel(x, routing_tensor, w0, b0, w1, b1):
    x = np.ascontiguousarray(np.asarray(x, dtype=np.float32))
    routing = np.asarray(routing_tensor, dtype=np.float32)
    w0 = np.asarray(w0, dtype=np.float32)
    b0 = np.asarray(b0, dtype=np.float32)
    w1 = np.asarray(w1, dtype=np.float32)
    b1 = np.asarray(b1, dtype=np.float32)

    idx = [np.nonzero(routing[:, e])[0] for e in range(E)]
    counts = [len(i) for i in idx]
    sizes, cores = _plan(counts)
    G = len(sizes)
    T = sum(sizes)
    goffs = np.concatenate([[0], np.cumsum(sizes)])

    nc = _cache.get(sizes)
    if nc is None:
        nc = _cache[sizes] = build_program(sizes)

    w0_bf = [np.ascontiguousarray(w0[e], dtype=ml_dtypes.bfloat16)
             for e in range(E)]
    w1_bf = [np.ascontiguousarray(w1[e], dtype=ml_dtypes.bfloat16)
             for e in range(E)]
    b0_cols = [np.ascontiguousarray(b0[e, 0].reshape(KF, P).T)
               for e in range(E)]

    in_maps = []
    for core in cores:
        xTc = np.zeros((D, T), dtype=ml_dtypes.bfloat16)
        b0c = np.empty((P, G * KF), dtype=np.float32)
        m = {"xT": xTc, "b0": b0c}
        for g, (e, lo, cnt) in enumerate(core):
            tok = idx[e][lo:lo + cnt]
            xTc[:, goffs[g]:goffs[g] + cnt] = \
                x[tok].T.astype(ml_dtypes.bfloat16)
            b0c[:, g * KF:(g + 1) * KF] = b0_cols[e]
            m[f"w0_{g}"] = w0_bf[e]
            m[f"w1_{g}"] = w1_bf[e]
        in_maps.append(m)

    res = run_bass_kernel_spmd(nc, in_maps, core_ids=list(range(8)))

    # combine: out = routing @ b1 + sum of r_e-scaled group outputs
    out = routing @ b1[:, 0, :]
    for ci, core in enumerate(cores):
        yT = res.results[ci]["yT"]
        for g, (e, lo, cnt) in enumerate(core):
            if cnt == 0:
                continue
            tok = idx[e][lo:lo + cnt]
            out[tok] += routing[tok, e:e + 1] * yT[:, goffs[g]:goffs[g] + cnt].T
    return out.astype(np.float32)


# revision 5
# speedup vs baseline: 1.1688x; 1.0345x over previous
"""MoE BatchedExperts kernel for 8 trn2 NeuronCores.

Strategy: expert parallelism with host-side top-k dispatch and exact load
balancing. Each token has TOP_K=2 nonzero routing weights; core c processes
a fixed per-core "slot structure" of expert token groups chosen so all
cores get ~N*K/E tokens (the hot experts are split across cores). All
matmuls run bf16 (1 row/cycle, same as fp32r, but half the DMA/SBUF and no
min-moving-dim constraint), PSUM accumulates fp32; measured end-to-end
rel err ~3e-3 vs the fp64 reference (gate 2e-2).

Per core, per group g (tokens gathered+transposed on host to xT [D, S_g]):
  h  = gelu(w0_g^T-tiles @ xT + b0)   [F-part, S_g]  tokens on moving dim
  yT = w1_g-tiles @ h                 [D-part, S_g]  tokens on moving dim
Host combines: out[idx] += r * yT.T rows; b1 folded in via routing @ b1.

Tokens stay on the PE moving dim in both phases so group sizes need no
128-padding. A few zero-filled warmup matmuls keep the PE busy (and ramp
its DVFS p-state) while the first input DMAs land.
"""

import numpy as np
import ml_dtypes

import concourse.bacc as bacc
import concourse.mybir as mybir
from concourse.tile import TileContext
from concourse.bass_utils import run_bass_kernel_spmd

F32 = mybir.dt.float32
BF16 = mybir.dt.bfloat16

N, D, E, F = 4096, 1024, 8, 2048
P = 128
KD = D // P            # 8  k-tiles for mm1 (contract D)
KF = F // P            # 16 k-tiles for mm2 (contract F)
DO = D // P            # 8  output d-tiles for mm2

_cache: dict[tuple, object] = {}


def _chunks_of(size: int) -> list[int]:
    """Split a group into near-equal moving-dim chunks <=512 (>=~250 keeps
    the per-matmul LDWEIGHTS (~97ns) hidden behind the previous matmul)."""
    n = -(-size // 512)
    base, rem = divmod(size, n)
    return [base + 1] * rem + [base] * (n - rem)


def build_program(sizes: tuple[int, ...]):
    """Bass program for one core: len(sizes) expert groups of fixed widths."""
    G = len(sizes)
    T = sum(sizes)
    goffs = [0, *np.cumsum(sizes).tolist()]
    chunks = [_chunks_of(s) for s in sizes]

    nc = bacc.Bacc("TRN2", target_bir_lowering=False, debug=False)
    xT = nc.dram_tensor("xT", [D, T], BF16, kind="ExternalInput")
    w0d = [nc.dram_tensor(f"w0_{g}", [D, F], BF16, kind="ExternalInput")
           for g in range(G)]
    w1d = [nc.dram_tensor(f"w1_{g}", [F, D], BF16, kind="ExternalInput")
           for g in range(G)]
    # b0 pre-arranged [128, G*KF] on the host (contiguous per-partition rows)
    b0 = nc.dram_tensor("b0", [P, G * KF], F32, kind="ExternalInput")
    yT = nc.dram_tensor("yT", [D, T], F32, kind="ExternalOutput")

    xT_r = xT.rearrange("(ko p) t -> p ko t", p=P)
    w0_r = [w.rearrange("(ko p) f -> p ko f", p=P) for w in w0d]
    w1_r = [w.rearrange("(ko p) d -> p ko d", p=P) for w in w1d]

    with TileContext(nc) as tc:
        with tc.tile_pool(name="const", bufs=1) as const, \
             tc.tile_pool(name="xpool", bufs=1) as xpool, \
             tc.tile_pool(name="hpool", bufs=1) as hpool, \
             tc.tile_pool(name="wpool", bufs=G + 1) as wpool, \
             tc.tile_pool(name="ypool", bufs=3) as ypool, \
             tc.tile_pool(name="psum", bufs=8, space="PSUM") as psum:

            # ---- PE warmup: zero matmuls ramp the DVFS p-state while the
            # first input DMAs are still in flight ----
            warm = const.tile([P, 512], BF16, name="warm")
            nc.vector.memset(warm[:], 0.0)
            for i in range(9):
                pw = psum.tile([P, 512], F32, tag="ps", name=f"warm{i}")
                nc.tensor.matmul(pw, warm[:, 0:P], warm[:], start=True,
                                 stop=True)

            # ---- input DMAs ----
            # scalar queue: ONLY x chunks + b0 (the 48 gelu ACTs also run on
            # the scalar engine; keeping bulk-weight descriptor pumping off
            # this queue keeps ACT drains flowing so PSUM banks never clog)
            x_sb = xpool.tile([P, KD, T], BF16, name="x")
            b0_sb = const.tile([P, G * KF], F32, name="b0")
            first = True
            for g in range(G):
                off = goffs[g]
                for c in chunks[g]:
                    nc.scalar.dma_start(x_sb[:, :, off:off + c],
                                        xT_r[:, :, off:off + c])
                    off += c
                    if first:
                        nc.scalar.dma_start(b0_sb[:], b0[:, :])
                        first = False

            # sync queue, strict priority order: w0 g0 (sliced so the first
            # fo tiles land fast), w0 g1, then w1 (needed only in phase 2),
            # then the y stores. w0/w1 tiles share G+1 slots (32KB each):
            # w1 of the last group reuses the slot w0 of group 0 releases
            # after phase-1.
            w0_sb, w1_sb = [], []
            for g in range(G):
                w = wpool.tile([P, KD, F], BF16, tag="wbig", name=f"w0_{g}")
                w0_sb.append(w)
                fo_slices = ([0, 256, 1024, 2048] if g == 0
                             else [0, 1024, 2048])
                for a, b in zip(fo_slices, fo_slices[1:]):
                    nc.sync.dma_start(w[:, :, a:b], w0_r[g][:, :, a:b])
            for g in range(G):
                w = wpool.tile([P, KF, D], BF16, tag="wbig", name=f"w1_{g}")
                w1_sb.append(w)
                for a in (0, 512):
                    nc.sync.dma_start(w[:, :, a:a + 512],
                                      w1_r[g][:, :, a:a + 512])

            # h = gelu(x @ w0 + b0), [F-part, T-free], groups concatenated
            h_sb = hpool.tile([P, KF, T], BF16, name="h")

            # ---- phase 1: mm1 + gelu ----
            for g in range(G):
                off = goffs[g]
                for c in chunks[g]:
                    for fo in range(KF):
                        ps = psum.tile([P, 512], F32, tag="ps",
                                       name=f"ps1_{g}_{off}_{fo}")[:, :c]
                        for k in range(KD):
                            nc.tensor.matmul(
                                ps, w0_sb[g][:, k, fo * P:(fo + 1) * P],
                                x_sb[:, k, off:off + c],
                                start=(k == 0), stop=(k == KD - 1))
                        nc.scalar.activation(
                            h_sb[:, fo, off:off + c], ps,
                            mybir.ActivationFunctionType.Gelu,
                            bias=b0_sb[:, g * KF + fo:g * KF + fo + 1])
                    off += c

            # ---- phase 2: mm2 ----
            for g in range(G):
                off = goffs[g]
                for c in chunks[g]:
                    for do in range(DO):
                        ps2 = psum.tile([P, 512], F32, tag="ps",
                                        name=f"ps2_{g}_{off}_{do}")[:, :c]
                        for k in range(KF):
                            nc.tensor.matmul(
                                ps2, w1_sb[g][:, k, do * P:(do + 1) * P],
                                h_sb[:, k, off:off + c],
                                start=(k == 0), stop=(k == KF - 1))
                        y_sb = ypool.tile([P, 512], F32, tag="y",
                                          name=f"y_{g}_{off}_{do}")[:, :c]
                        nc.vector.tensor_copy(y_sb, ps2)
                        nc.sync.dma_start(
                            yT[do * P:(do + 1) * P, off:off + c], y_sb)
                    off += c

    nc.compile()
    return nc


def _plan(counts):
    """Choose per-core slot sizes (S1, S2) and assign expert token pieces.

    Minimizes T = S1 + S2 such that the 8 experts can be covered by 8
    pieces of size <= S1 plus 8 of size <= S2 (pieces of one expert may
    live on different cores). Falls back to one-slot-per-core (pure expert
    parallelism) if the search fails.
    """
    cmax = int(max(counts))
    order = sorted(range(E), key=lambda e: -counts[e])
    csort = [int(counts[e]) for e in order]

    def assign(S1, S2):
        from functools import lru_cache

        @lru_cache(maxsize=None)
        def feas(i, a, b):
            if i == len(csort):
                return ()
            c = csort[i]
            opts = []
            if c <= S1: opts.append((1, 0))
            if c <= S2: opts.append((0, 1))
            if c <= 2 * S2: opts.append((0, 2))
            if c <= S1 + S2: opts.append((1, 1))
            if c <= 2 * S1: opts.append((2, 0))
            if c <= S1 + 2 * S2: opts.append((1, 2))
            if c <= 2 * S1 + S2: opts.append((2, 1))
            opts.sort(key=lambda uv: (uv[0] + uv[1], S1 * uv[0] + S2 * uv[1]))
            for u, v in opts:
                if u <= a and v <= b:
                    rest = feas(i + 1, a - u, b - v)
                    if rest is not None:
                        return ((u, v),) + rest
            return None

        return feas(0, 8, 8)

    best = None
    for T in range(-(-N * 2 // E), cmax + 1):
        for S1 in range(-(-T // 2), T):
            S2 = T - S1
            sol = assign(S1, S2)
            if sol is not None:
                best = (S1, S2, sol)
                break
        if best:
            break
    if best is None:
        sizes = (cmax,)
        cores = [[(e, 0, int(counts[e]))] for e in range(E)]
        return sizes, cores

    S1, S2, sol = best
    s1_pieces, s2_pieces = [], []
    for i, (u, v) in enumerate(sol):
        e, c = order[i], csort[i]
        caps = [S1] * u + [S2] * v
        lo_ = 0
        for j, cap in enumerate(caps):
            take = min(cap, c - lo_)
            # ensure later pieces aren't left with more than they can hold
            take = max(take, c - lo_ - sum(caps[j + 1:]))
            (s1_pieces if cap == S1 else s2_pieces).append((e, lo_, take))
            lo_ += take
    while len(s1_pieces) < 8:
        s1_pieces.append((0, 0, 0))
    while len(s2_pieces) < 8:
        s2_pieces.append((0, 0, 0))
    sizes = (S1, S2)
    cores = [[s1_pieces[i], s2_pieces[i]] for i in range(8)]
    return sizes, cores


def kernel(x, routing_tensor, w0, b0, w1, b1):
    x = np.ascontiguousarray(np.asarray(x, dtype=np.float32))
    routing = np.asarray(routing_tensor, dtype=np.float32)
    w0 = np.asarray(w0, dtype=np.float32)
    b0 = np.asarray(b0, dtype=np.float32)
    w1 = np.asarray(w1, dtype=np.float32)
    b1 = np.asarray(b1, dtype=np.float32)

    idx = [np.nonzero(routing[:, e])[0] for e in range(E)]
    counts = [len(i) for i in idx]
    sizes, cores = _plan(counts)
    G = len(sizes)
    T = sum(sizes)
    goffs = np.concatenate([[0], np.cumsum(sizes)])

    nc = _cache.get(sizes)
    if nc is None:
        nc = _cache[sizes] = build_program(sizes)

    w0_bf = [np.ascontiguousarray(w0[e], dtype=ml_dtypes.bfloat16)
             for e in range(E)]
    w1_bf = [np.ascontiguousarray(w1[e], dtype=ml_dtypes.bfloat16)
             for e in range(E)]
    b0_cols = [np.ascontiguousarray(b0[e, 0].reshape(KF, P).T)
               for e in range(E)]

    in_maps = []
    for core in cores:
        xTc = np.zeros((D, T), dtype=ml_dtypes.bfloat16)
        b0c = np.empty((P, G * KF), dtype=np.float32)
        m = {"xT": xTc, "b0": b0c}
        for g, (e, lo, cnt) in enumerate(core):
            tok = idx[e][lo:lo + cnt]
            xTc[:, goffs[g]:goffs[g] + cnt] = \
                x[tok].T.astype(ml_dtypes.bfloat16)
            b0c[:, g * KF:(g + 1) * KF] = b0_cols[e]
            m[f"w0_{g}"] = w0_bf[e]
            m[f"w1_{g}"] = w1_bf[e]
        in_maps.append(m)

    res = run_bass_kernel_spmd(nc, in_maps, core_ids=list(range(8)))

    # combine: out = routing @ b1 + sum of r_e-scaled group outputs
    out = routing @ b1[:, 0, :]
    for ci, core in enumerate(cores):
        yT = res.results[ci]["yT"]
        for g, (e, lo, cnt) in enumerate(core):
            if cnt == 0:
                continue
            tok = idx[e][lo:lo + cnt]
            out[tok] += routing[tok, e:e + 1] * yT[:, goffs[g]:goffs[g] + cnt].T
    return out.astype(np.float32)
